# revision 34
# baseline (speedup 1.0000x reference)
"""DeepSpeed-style MLP block (residual-add + LayerNorm + GEMM + GeLU + GEMM +
residual) on 8 Trainium2 NeuronCores.

Sharding: data-parallel over tokens (B*S = 8192 -> 1024 tokens/core).  Each
core holds the full weights and computes its token slice end-to-end; no
collectives.  All matmuls run in fp32 on the PE (lhsT.T @ rhs, contraction on
the partition dim), so activations are transposed once after LayerNorm via PE
transposes ([H, tok] layout), the first GEMM produces h^T = [I, tok] tiles
(gelu applied on the PSUM->SBUF drain), and the second GEMM consumes h^T
chunks as the stationary operand against output_w rows, yielding out[tok, H]
naturally.
"""

import sys

sys.path.insert(0, "/opt/trn_rl_repo")

import numpy as np

try:
    import jax

    jax.config.update("jax_compilation_cache_dir", "/tmp/jax_neff_cache")
    jax.config.update("jax_persistent_cache_min_compile_time_secs", 1.0)
    jax.config.update("jax_persistent_cache_min_entry_size_bytes", 0)
except Exception:
    pass

import concourse.bass as bass  # noqa: F401  (engine types referenced via nc)
import concourse.mybir as mybir
from concourse import bacc
from concourse.masks import make_identity
from concourse.tile import TileContext

F32 = mybir.dt.float32
AF = mybir.ActivationFunctionType
N_CORES = 8
B, S, H, I = 4, 2048, 2048, 8192
LN_EPS = 1e-6
NTOK = B * S                 # 8192 tokens total
TLOC = NTOK // N_CORES       # 1024 tokens per core
TT = TLOC // 128             # 8 token tiles per core
HC = H // 128                # 16 eta (hidden) chunks
IC = I // 128                # 64 i chunks
OC = H // 512                # 4 output column chunks of 512

_CACHE = {}


def _build_program():
    nc = bacc.Bacc("TRN2", target_bir_lowering=False, debug=False,
                   num_devices=N_CORES)

    xin = nc.declare_dram_parameter("xin", [TLOC, H], F32, isOutput=False)
    xres = nc.declare_dram_parameter("xres", [TLOC, H], F32, isOutput=False)
    # w1p[i, c, p, f] = inter_w[c*128 + p, i*128 + f]
    w1p = nc.declare_dram_parameter("w1p", [IC, HC, 128, 128], F32, isOutput=False)
    w2p = nc.declare_dram_parameter("w2p", [I, H], F32, isOutput=False)
    biasb = nc.declare_dram_parameter("biasb", [128, H], F32, isOutput=False)
    obb = nc.declare_dram_parameter("obb", [128, H], F32, isOutput=False)
    gammat = nc.declare_dram_parameter("gammat", [128, HC], F32, isOutput=False)
    betat = nc.declare_dram_parameter("betat", [128, HC], F32, isOutput=False)
    ibt = nc.declare_dram_parameter("ibt", [128, IC], F32, isOutput=False)
    out = nc.declare_dram_parameter("out", [TLOC, H], F32, isOutput=True)

    with TileContext(nc) as tc:
        with (
            tc.tile_pool(name="perm", bufs=1) as perm,
            tc.tile_pool(name="dram", bufs=1, space="DRAM") as dpool,
        ):
            ident = perm.tile([128, 128], F32)
            make_identity(nc, ident[:])
            eps = perm.tile([128, 1], F32)
            nc.vector.memset(eps[:], LN_EPS)
            gam = perm.tile([128, HC], F32)
            bet = perm.tile([128, HC], F32)
            ib = perm.tile([128, IC], F32)
            nc.sync.dma_start(out=gam[:], in_=gammat[:])
            nc.sync.dma_start(out=bet[:], in_=betat[:])
            nc.sync.dma_start(out=ib[:], in_=ibt[:])

            # residual_add tiles stay resident for the final add
            ras = [perm.tile([128, H], F32, name=f"ra{t}") for t in range(TT)]
            hts_dram = [dpool.tile([128, TLOC], F32, name=f"htd{i}")
                        for i in range(IC)]

            with tc.tile_pool(name="p12", bufs=1) as p12:
                # ln^T resident: one [128, TLOC] tile per eta chunk
                lnt = [p12.tile([128, TLOC], F32, name=f"lnt{c}")
                       for c in range(HC)]

                # ---------------- phase 1: residual add + LN + transpose ----
                with (
                    tc.tile_pool(name="p1", bufs=2) as p1,
                    tc.tile_pool(name="p1c", bufs=1) as p1c,
                    tc.tile_pool(name="trp", bufs=4, space="PSUM") as trp,
                ):
                    bb = p1c.tile([128, H], F32)
                    nc.sync.dma_start(out=bb[:], in_=biasb[:])
                    for t in range(TT):
                        tin = p1.tile([128, H], F32, tag="tin")
                        tre = p1.tile([128, H], F32, tag="tre")
                        nc.sync.dma_start(out=tin[:], in_=xin[t * 128:(t + 1) * 128, :])
                        nc.sync.dma_start(out=tre[:], in_=xres[t * 128:(t + 1) * 128, :])
                        ra = ras[t]
                        nc.vector.tensor_add(ra[:], tin[:], tre[:])
                        nc.vector.tensor_add(ra[:], ra[:], bb[:])
                        # stats
                        scr = p1.tile([128, H], F32, tag="scr")
                        ssq = p1.tile([128, 1], F32, tag="ssq")
                        nc.scalar.activation(scr[:], ra[:], AF.Square,
                                             accum_out=ssq[:])
                        s1 = p1.tile([128, 1], F32, tag="s1")
                        nc.vector.reduce_sum(s1[:], ra[:], axis=mybir.AxisListType.X)
                        mu = p1.tile([128, 1], F32, tag="mu")
                        nc.vector.tensor_scalar_mul(mu[:], s1[:], 1.0 / H)
                        ex2 = p1.tile([128, 1], F32, tag="ex2")
                        nc.vector.tensor_scalar_mul(ex2[:], ssq[:], 1.0 / H)
                        mu2 = p1.tile([128, 1], F32, tag="mu2")
                        nc.vector.tensor_mul(mu2[:], mu[:], mu[:])
                        var = p1.tile([128, 1], F32, tag="var")
                        nc.vector.tensor_sub(var[:], ex2[:], mu2[:])
                        std = p1.tile([128, 1], F32, tag="std")
                        nc.scalar.activation(std[:], var[:], AF.Sqrt, bias=eps[:])
                        rstd = p1.tile([128, 1], F32, tag="rstd")
                        nc.vector.reciprocal(rstd[:], std[:])
                        z = p1.tile([128, H], F32, tag="scr")
                        nc.vector.tensor_scalar(
                            z[:], ra[:], mu[:], rstd[:],
                            op0=mybir.AluOpType.subtract,
                            op1=mybir.AluOpType.mult,
                        )
                        # transpose 128x128 blocks; gamma/beta on the drain
                        for c in range(HC):
                            ps = trp.tile([128, 128], F32, tag="tr")
                            nc.tensor.transpose(
                                ps[:], z[:, c * 128:(c + 1) * 128], ident[:])
                            nc.scalar.activation(
                                lnt[c][:, t * 128:(t + 1) * 128], ps[:],
                                AF.Identity,
                                bias=bet[:, c:c + 1], scale=gam[:, c:c + 1])

                # ---------------- phase 2: h^T = gelu(W1^T @ ln^T + b1) -----
                with (
                    tc.tile_pool(name="p2", bufs=3) as p2,
                    tc.tile_pool(name="pshp", bufs=4, space="PSUM") as pshp,
                ):
                    for i in range(IC):
                        w1t = p2.tile([128, H], F32, tag="w1t")
                        for c in range(HC):
                            nc.sync.dma_start(
                                out=w1t[:, c * 128:(c + 1) * 128],
                                in_=w1p[i, c])
                        ht = p2.tile([128, TLOC], F32, tag="ht")
                        for half in range(TLOC // 512):
                            psh = pshp.tile([128, 512], F32, tag="psh")
                            for c in range(HC):
                                nc.tensor.matmul(
                                    psh[:],
                                    w1t[:, c * 128:(c + 1) * 128],
                                    lnt[c][:, half * 512:(half + 1) * 512],
                                    start=(c == 0), stop=(c == HC - 1))
                            nc.scalar.activation(
                                ht[:, half * 512:(half + 1) * 512], psh[:],
                                AF.Gelu, bias=ib[:, i:i + 1])
                        nc.sync.dma_start(out=hts_dram[i][:], in_=ht[:])

            # ---------------- phase 3: out = h @ W2 + ra + b_out ------------
            with (
                tc.tile_pool(name="p3", bufs=1) as p3c,
                tc.tile_pool(name="p3w", bufs=6) as p3w,
                tc.tile_pool(name="p3h", bufs=10) as p3h,
                tc.tile_pool(name="p3o", bufs=3) as p3o,
                tc.tile_pool(name="psop", bufs=1, space="PSUM") as psop,
            ):
                ob = p3c.tile([128, H], F32)
                nc.sync.dma_start(out=ob[:], in_=obb[:])
                for pair in range(TT // 2):
                    psos = [psop.tile([128, H], F32, name=f"pso{pair}_{k}",
                                      tag=f"pso{k}") for k in range(2)]
                    for i in range(IC):
                        w2t = p3w.tile([128, H], F32, tag="w2t")
                        nc.sync.dma_start(
                            out=w2t[:], in_=w2p[i * 128:(i + 1) * 128, :])
                        for k in range(2):
                            t = pair * 2 + k
                            htt = p3h.tile([128, 128], F32, tag="htt")
                            nc.sync.dma_start(
                                out=htt[:],
                                in_=hts_dram[i][:, t * 128:(t + 1) * 128])
                            for o in range(OC):
                                nc.tensor.matmul(
                                    psos[k][:, o * 512:(o + 1) * 512],
                                    htt[:],
                                    w2t[:, o * 512:(o + 1) * 512],
                                    start=(i == 0), stop=(i == IC - 1))
                    for k in range(2):
                        t = pair * 2 + k
                        osb = p3o.tile([128, H], F32, tag="osb")
                        nc.vector.tensor_add(osb[:], psos[k][:], ras[t][:])
                        nc.vector.tensor_add(osb[:], osb[:], ob[:])
                        nc.sync.dma_start(
                            out=out[t * 128:(t + 1) * 128, :], in_=osb[:])

    nc.compile()
    return nc


def _get_program():
    if "nc" not in _CACHE:
        _CACHE["nc"] = _build_program()
    return _CACHE["nc"]


def kernel(input, residual, residual_norm, bias, gamma, beta,
           inter_w, inter_b, output_w, output_b):
    nc = _get_program()

    input = np.ascontiguousarray(np.asarray(input, dtype=np.float32))
    residual = np.ascontiguousarray(np.asarray(residual, dtype=np.float32))
    bias = np.asarray(bias, dtype=np.float32)
    gamma = np.asarray(gamma, dtype=np.float32)
    beta = np.asarray(beta, dtype=np.float32)
    inter_w = np.asarray(inter_w, dtype=np.float32)
    inter_b = np.asarray(inter_b, dtype=np.float32)
    output_w = np.ascontiguousarray(np.asarray(output_w, dtype=np.float32))
    output_b = np.asarray(output_b, dtype=np.float32)

    xin = input.reshape(NTOK, H)
    xres = residual.reshape(NTOK, H)
    # w1p[i, c, p, f] = inter_w[c*128+p, i*128+f]
    w1p = np.ascontiguousarray(
        inter_w.reshape(HC, 128, IC, 128).transpose(2, 0, 1, 3))
    biasb = np.ascontiguousarray(np.broadcast_to(bias, (128, H)))
    obb = np.ascontiguousarray(np.broadcast_to(output_b, (128, H)))
    gammat = np.ascontiguousarray(gamma.reshape(HC, 128).T)
    betat = np.ascontiguousarray(beta.reshape(HC, 128).T)
    ibt = np.ascontiguousarray(inter_b.reshape(IC, 128).T)

    in_maps = []
    for c in range(N_CORES):
        in_maps.append({
            "xin": np.ascontiguousarray(xin[c * TLOC:(c + 1) * TLOC]),
            "xres": np.ascontiguousarray(xres[c * TLOC:(c + 1) * TLOC]),
            "w1p": w1p,
            "w2p": output_w,
            "biasb": biasb,
            "obb": obb,
            "gammat": gammat,
            "betat": betat,
            "ibt": ibt,
        })

    from concourse.bass_utils import run_bass_kernel_spmd
    res = run_bass_kernel_spmd(nc, in_maps, list(range(N_CORES)))
    out = np.concatenate([res.results[c]["out"] for c in range(N_CORES)], axis=0)
    return out.reshape(B, S, H)


if __name__ == "__main__":
    nc = _get_program()
    from concourse.timeline_sim import TimelineSim
    ts = TimelineSim(nc)
    total = ts.simulate()
    print(f"TimelineSim: {total:.0f} ns")
